# revision 1
# baseline (speedup 1.0000x reference)
import sys, time
sys.path.insert(0, "/opt/trn_rl_repo")
import numpy as np
import ml_dtypes
from contextlib import ExitStack

import concourse.bass as bass
import concourse.tile as tile
from concourse import mybir, bacc
from concourse.bass_utils import run_bass_kernel_spmd

BF16 = ml_dtypes.bfloat16
F32 = mybir.dt.float32
BF = mybir.dt.bfloat16
AF = mybir.ActivationFunctionType
OP = mybir.AluOpType

B, L, DM, ED, EDH, N, DT_RANK, NL = 4, 1024, 512, 1024, 512, 16, 32, 2
EPS = 1e-5
RG = [[0, 1], [2, 3], [4, 5], [6, 7]]

REPEAT = 1
LAST_RUN_S = 0.0
ABLATE = frozenset()
_CACHE = {}


def _build(repeat, a_li, mode=frozenset()):
    ndev = 1 if "single" in mode else 8
    nc = bacc.Bacc("TRN2", target_bir_lowering=False, debug=False, num_devices=ndev)
    xT_d = nc.dram_tensor("xT", [128, 4096], F32, kind="ExternalInput")
    winT_d = nc.dram_tensor("winT", [128, 8192], BF, kind="ExternalInput")
    cvd_d = nc.dram_tensor("cvd", [128, 4096], BF, kind="ExternalInput")
    convb_d = nc.dram_tensor("convb", [128, 8], F32, kind="ExternalInput")
    wxp_d = nc.dram_tensor("wxp", [128, 512], BF, kind="ExternalInput")
    wdt_d = nc.dram_tensor("wdt", [32, 1024], BF, kind="ExternalInput")
    dtb_d = nc.dram_tensor("dtb", [128, 8], F32, kind="ExternalInput")
    Dv_d = nc.dram_tensor("Dv", [128, 8], F32, kind="ExternalInput")
    wout_d = nc.dram_tensor("wout", [128, 4096], BF, kind="ExternalInput")
    fcp_d = nc.dram_tensor("fcp", [128, 4], BF, kind="ExternalInput")
    selp_d = nc.dram_tensor("selp", [64, 4096], BF, kind="ExternalInput")
    fcb_d = nc.dram_tensor("fcb", [1, 1], F32, kind="ExternalInput")
    out_d = nc.dram_tensor("out", [1, 1024], F32, kind="ExternalOutput")
    cc = {}
    for li in range(2):
        cc[("dbc_in", li)] = nc.dram_tensor(f"ccdbci{li}", [64, 1024], BF)
        cc[("dbc_out", li)] = nc.dram_tensor(f"ccdbco{li}", [64, 1024], BF)
        cc[("bo_in", li)] = nc.dram_tensor(f"ccboi{li}", [128, 4096], BF)
        cc[("bo_out", li)] = nc.dram_tensor(f"ccboo{li}", [128, 4096], BF)

    with tile.TileContext(nc) as tc, ExitStack() as ctx:
        sb = ctx.enter_context(tc.tile_pool(name="sb", bufs=1))
        wk = ctx.enter_context(tc.tile_pool(name="wk", bufs=2))
        pp = ctx.enter_context(
            tc.tile_pool(name="pp", bufs=4, space=bass.MemorySpace.PSUM))

        def ps():
            t = pp.tile([128, 1024], F32, name="ps", tag="ps")
            return t

        winT_s = sb.tile([128, 8192], BF)
        nc.sync.dma_start(winT_s[:], winT_d[:])
        cvd_s = sb.tile([128, 4096], BF)
        nc.sync.dma_start(cvd_s[:], cvd_d[:])
        convb_s = sb.tile([128, 8], F32)
        nc.sync.dma_start(convb_s[:], convb_d[:])
        wxp_s = sb.tile([128, 512], BF)
        nc.sync.dma_start(wxp_s[:], wxp_d[:])
        wdt_s = sb.tile([32, 1024], BF)
        nc.sync.dma_start(wdt_s[:], wdt_d[:])
        dtb_s = sb.tile([128, 8], F32)
        nc.sync.dma_start(dtb_s[:], dtb_d[:])
        Dv_s = sb.tile([128, 8], F32)
        nc.sync.dma_start(Dv_s[:], Dv_d[:])
        wout_s = sb.tile([128, 4096], BF)
        nc.sync.dma_start(wout_s[:], wout_d[:])
        fcp_s = sb.tile([128, 4], BF)
        nc.sync.dma_start(fcp_s[:], fcp_d[:])
        selp_s = sb.tile([64, 4096], BF)
        nc.sync.dma_start(selp_s[:], selp_d[:])
        fcb_s = sb.tile([1, 1], F32)
        nc.sync.dma_start(fcb_s[:], fcb_d[:])
        onescol = sb.tile([128, 1], BF)
        nc.vector.memset(onescol[:], 1.0)
        onesP = sb.tile([128, 128], BF)
        nc.vector.memset(onesP[:], 1.0)
        epsc = sb.tile([1, 1], F32)
        nc.vector.memset(epsc[:], EPS)

        xT_s = sb.tile([128, 4096], F32)
        y_acc = sb.tile([128, 4096], F32)
        xn_s = sb.tile([128, 4096], BF)
        pxin_s = sb.tile([128, 4096], BF)
        sz_s = sb.tile([128, 4096], BF)
        xin_s = sb.tile([128, 4096], BF)
        delta_s = sb.tile([128, 4096], BF)
        u_s = sb.tile([128, 4096], BF)  # also rmsnorm sq / head bf16 scratch
        y3_s = sb.tile([128, 4096], BF)
        bo_s = sb.tile([128, 4096], BF)
        dbc_s = sb.tile([64, 1024], BF)

        MM = nc.tensor.matmul
        ACT = nc.scalar.activation
        TT = nc.vector.tensor_tensor

        for _r in range(repeat):
            nc.sync.dma_start(xT_s[:], xT_d[:])
            for li in range(NL):
                a_n = a_li[li]
                # ---- rmsnorm ----
                ACT(u_s[:], xT_s[:], AF.Square)
                mps = ps()
                for th in range(2):
                    for dc in range(4):
                        MM(mps[0:1, th * 512:(th + 1) * 512], onescol[:, 0:1],
                           u_s[:, dc * 1024 + th * 512: dc * 1024 + th * 512 + 512],
                           start=(dc == 0), stop=(dc == 3))
                ln_t = wk.tile([1, 1024], F32, bufs=1)
                ACT(ln_t[:], mps[0:1, 0:1024], AF.Ln, scale=1.0 / DM, bias=epsc[:])
                rstd_s = wk.tile([1, 1024], BF, bufs=1)
                ACT(rstd_s[:], ln_t[:], AF.Exp, scale=-0.5)
                rbp = ps()
                for th in range(2):
                    MM(rbp[:, th * 512:(th + 1) * 512], onesP[0:1, :],
                       rstd_s[0:1, th * 512:(th + 1) * 512], start=True, stop=True)
                for dc in range(4):
                    TT(xn_s[:, dc * 1024:(dc + 1) * 1024],
                       xT_s[:, dc * 1024:(dc + 1) * 1024], rbp[:, 0:1024], OP.mult)

                # ---- in_proj (xin | z) ----
                for grp in range(2):
                    for co in range(4):
                        pin = ps()
                        for th in range(2):
                            for dc in range(4):
                                off = li * 4096 + grp * 2048 + co * 512 + dc * 128
                                MM(pin[:, th * 512:(th + 1) * 512],
                                   winT_s[:, off:off + 128],
                                   xn_s[:, dc * 1024 + th * 512: dc * 1024 + th * 512 + 512],
                                   start=(dc == 0), stop=(dc == 3))
                        if grp == 0:
                            ACT(pxin_s[:, co * 1024:(co + 1) * 1024], pin[:, 0:1024], AF.Copy)
                        else:
                            ACT(sz_s[:, co * 1024:(co + 1) * 1024], pin[:, 0:1024], AF.Silu)

                # ---- causal depthwise conv + silu ----
                for c in range(4):
                    pc = ps()
                    base = c * 1024
                    for th in range(2):
                        for s in range(4):  # shift = 3 - tap
                            k = 3 - s
                            w0 = li * 2048 + c * 512 + k * 128
                            lh = cvd_s[:, w0:w0 + 128]
                            if th == 0:
                                o0, o1, i0 = s, 512, base
                            else:
                                o0, o1, i0 = 512, 1024, base + 512 - s
                            MM(pc[:, o0:o1], lh, pxin_s[:, i0:i0 + (o1 - o0)],
                               start=(s == 0), stop=(s == 3), skip_group_check=True)
                    ACT(xin_s[:, base:base + 1024], pc[:, 0:1024], AF.Silu,
                        bias=convb_s[:, li * 4 + c: li * 4 + c + 1])

                # ---- x_proj partial + pair AllReduce ----
                pxp = ps()
                for th in range(2):
                    for c in range(4):
                        MM(pxp[0:64, th * 512:(th + 1) * 512],
                           wxp_s[:, li * 256 + c * 64: li * 256 + (c + 1) * 64],
                           xin_s[:, c * 1024 + th * 512: c * 1024 + th * 512 + 512],
                           start=(c == 0), stop=(c == 3))
                dbc_l = wk.tile([64, 1024], BF, bufs=1)
                ACT(dbc_l[:], pxp[0:64, 0:1024], AF.Copy)
                nc.sync.dma_start(cc[("dbc_in", li)][:], dbc_l[:])
                if "nocc" in mode:
                    nc.sync.dma_start(cc[("dbc_out", li)][:], cc[("dbc_in", li)][:])
                else:
                    nc.gpsimd.collective_compute(
                        "AllReduce", OP.add, ins=[cc[("dbc_in", li)][:]],
                        outs=[cc[("dbc_out", li)][:]], replica_groups=RG)
                nc.sync.dma_start(dbc_s[:], cc[("dbc_out", li)][:])

                # ---- delta = softplus(dt proj), u = delta*xin ----
                for c in range(4):
                    pd = ps()
                    for th in range(2):
                        MM(pd[:, th * 512:(th + 1) * 512],
                           wdt_s[0:32, li * 512 + c * 128: li * 512 + (c + 1) * 128],
                           dbc_s[0:32, th * 512:(th + 1) * 512], start=True, stop=True)
                    pe = ps()
                    ACT(pe[:, 0:1024], pd[:, 0:1024], AF.Exp,
                        bias=dtb_s[:, li * 4 + c: li * 4 + c + 1])
                    ACT(delta_s[:, c * 1024:(c + 1) * 1024], pe[:, 0:1024],
                        AF.Ln, bias=1.0)
                TT(u_s[:], delta_s[:], xin_s[:], OP.mult)

                # ---- y_acc = D * xin ----
                for c in range(4):
                    ACT(y_acc[:, c * 1024:(c + 1) * 1024],
                        xin_s[:, c * 1024:(c + 1) * 1024], AF.Copy,
                        scale=Dv_s[:, li * 4 + c: li * 4 + c + 1])

                # ---- selective scan over n ----
                for n in range(0 if "noscan" not in mode else N, N):
                    pb = ps()
                    pcn = ps()
                    for th in range(2):
                        MM(pb[:, th * 512:(th + 1) * 512],
                           selp_s[32:64, n * 128:(n + 1) * 128],
                           dbc_s[32:64, th * 512:(th + 1) * 512],
                           start=True, stop=True)
                        MM(pcn[:, th * 512:(th + 1) * 512],
                           selp_s[32:64, (16 + n) * 128:(17 + n) * 128],
                           dbc_s[32:64, th * 512:(th + 1) * 512],
                           start=True, stop=True)
                    Bb = wk.tile([128, 1024], BF)
                    ACT(Bb[:], pb[:, 0:1024], AF.Copy)
                    Cb = wk.tile([128, 1024], BF)
                    ACT(Cb[:], pcn[:, 0:1024], AF.Copy)
                    dA = wk.tile([128, 4096], BF)
                    ACT(dA[:], delta_s[:], AF.Exp, scale=float(a_n[n]))
                    for c in range(4):
                        blk = slice(c * 1024, (c + 1) * 1024)
                        dBx = wk.tile([128, 1024], BF)
                        TT(dBx[:], u_s[:, blk], Bb[:], OP.mult)
                        hh = wk.tile([128, 1024], BF)
                        nc.vector.tensor_tensor_scan(
                            hh[:], dA[:, blk], dBx[:], 0.0, OP.mult, OP.add)
                        yn = wk.tile([128, 1024], BF)
                        TT(yn[:], hh[:], Cb[:], OP.mult)
                        TT(y_acc[:, blk], y_acc[:, blk], yn[:], OP.add)

                # ---- gate ----
                TT(y3_s[:], y_acc[:], sz_s[:], OP.mult)

                # ---- out_proj partial + pair AllReduce + residual ----
                for dc in range(4):
                    po = ps()
                    for th in range(2):
                        for c in range(4):
                            off = li * 2048 + c * 512 + dc * 128
                            MM(po[:, th * 512:(th + 1) * 512], wout_s[:, off:off + 128],
                               y3_s[:, c * 1024 + th * 512: c * 1024 + th * 512 + 512],
                               start=(c == 0), stop=(c == 3))
                    ACT(bo_s[:, dc * 1024:(dc + 1) * 1024], po[:, 0:1024], AF.Copy)
                nc.sync.dma_start(cc[("bo_in", li)][:], bo_s[:])
                if "nocc" in mode:
                    nc.sync.dma_start(cc[("bo_out", li)][:], cc[("bo_in", li)][:])
                else:
                    nc.gpsimd.collective_compute(
                        "AllReduce", OP.add, ins=[cc[("bo_in", li)][:]],
                        outs=[cc[("bo_out", li)][:]], replica_groups=RG)
                nc.sync.dma_start(bo_s[:], cc[("bo_out", li)][:])
                TT(xT_s[:], xT_s[:], bo_s[:], OP.add)

            # ---- head: logits + sigmoid ----
            ACT(u_s[:], xT_s[:], AF.Copy)
            pf = ps()
            for th in range(2):
                for dc in range(4):
                    MM(pf[0:1, th * 512:(th + 1) * 512], fcp_s[:, dc:dc + 1],
                       u_s[:, dc * 1024 + th * 512: dc * 1024 + th * 512 + 512],
                       start=(dc == 0), stop=(dc == 3))
            out_t = wk.tile([1, 1024], F32, bufs=1)
            ACT(out_t[:], pf[0:1, 0:1024], AF.Sigmoid, bias=fcb_s[0:1, 0:1])
            nc.sync.dma_start(out_d[:], out_t[:])

    nc.finalize()
    return nc


def _pack_core(inp, b, eh):
    sl = slice(eh * EDH, (eh + 1) * EDH)
    m = {}
    xt = np.asarray(inp["x"])[b].T.astype(np.float32)  # [512, 1024]
    m["xT"] = np.ascontiguousarray(
        xt.reshape(4, 128, 1024).transpose(1, 0, 2).reshape(128, 4096))
    winT = np.zeros((128, 8192), BF16)
    for li in range(NL):
        W = (np.asarray(inp["in_proj_w"])[li].astype(np.float32)
             * np.asarray(inp["norm_w"])[li][None, :].astype(np.float32))
        for grp, Wg in ((0, W[sl]), (1, W[ED + eh * EDH: ED + (eh + 1) * EDH])):
            WgT = Wg.T.astype(BF16)  # [512 k, 512 co]
            for co in range(4):
                for dc in range(4):
                    col = li * 4096 + grp * 2048 + co * 512 + dc * 128
                    winT[:, col:col + 128] = WgT[dc * 128:(dc + 1) * 128,
                                                 co * 128:(co + 1) * 128]
    m["winT"] = winT
    cvd = np.zeros((128, 4096), BF16)
    for li in range(NL):
        cw = np.asarray(inp["conv_w"])[li][:, 0, :][sl].astype(np.float32)  # [512,4]
        for c in range(4):
            for k in range(4):
                col = li * 2048 + c * 512 + k * 128
                cvd[:, col:col + 128] = np.diag(cw[c * 128:(c + 1) * 128, k]).astype(BF16)
    m["cvd"] = cvd

    def cols8(v):
        out = np.zeros((128, 8), np.float32)
        for li in range(NL):
            out[:, li * 4:(li + 1) * 4] = np.asarray(v)[li][sl].astype(
                np.float32).reshape(4, 128).T
        return out

    m["convb"] = cols8(inp["conv_b"])
    m["dtb"] = cols8(inp["dt_b"])
    m["Dv"] = cols8(inp["D"])
    wxp = np.zeros((128, 512), BF16)
    for li in range(NL):
        WxpT = np.asarray(inp["x_proj_w"])[li][:, sl].T.astype(BF16)  # [512, 64]
        for c in range(4):
            wxp[:, li * 256 + c * 64: li * 256 + (c + 1) * 64] = \
                WxpT[c * 128:(c + 1) * 128]
    m["wxp"] = wxp
    wdt = np.zeros((32, 1024), BF16)
    for li in range(NL):
        Wdt = np.asarray(inp["dt_w"])[li][sl].astype(BF16)  # [512, 32]
        for c in range(4):
            wdt[:, li * 512 + c * 128: li * 512 + (c + 1) * 128] = \
                Wdt[c * 128:(c + 1) * 128].T
    m["wdt"] = wdt
    wout = np.zeros((128, 4096), BF16)
    for li in range(NL):
        WoT = np.asarray(inp["out_proj_w"])[li][:, sl].T.astype(BF16)  # [512e,512dm]
        for c in range(4):
            for dc in range(4):
                col = li * 2048 + c * 512 + dc * 128
                wout[:, col:col + 128] = WoT[c * 128:(c + 1) * 128,
                                             dc * 128:(dc + 1) * 128]
    m["wout"] = wout
    fcp = np.zeros((128, 4), BF16)
    fw = np.asarray(inp["fc_w"]).reshape(-1).astype(BF16)
    for dc in range(4):
        fcp[:, dc] = fw[dc * 128:(dc + 1) * 128]
    m["fcp"] = fcp
    m["fcb"] = np.array([[float(np.asarray(inp["fc_b"]).reshape(-1)[0])]], np.float32)
    selp = np.zeros((64, 4096), BF16)
    for n in range(N):
        selp[32 + n, n * 128:(n + 1) * 128] = 1.0       # pick B_n row
        selp[48 + n, (16 + n) * 128:(17 + n) * 128] = 1.0  # pick C_n row
    m["selp"] = selp
    return m


def kernel(**inputs):
    global LAST_RUN_S
    a_li = []
    for li in range(NL):
        A = -np.exp(np.asarray(inputs["A_log"])[li].astype(np.float64))  # [ED, N]
        a0 = A[0]
        assert np.abs(A - a0[None, :]).max() <= 1e-6 * np.abs(a0).max(), \
            "A not uniform across channels"
        a_li.append(tuple(float(v) for v in a0))
    key = (REPEAT, ABLATE, a_li[0], a_li[1])
    if key not in _CACHE:
        _CACHE[key] = _build(REPEAT, a_li, ABLATE)
    nc = _CACHE[key]
    in_maps = [_pack_core(inputs, core // 2, core % 2) for core in range(8)]
    t0 = time.time()
    res = run_bass_kernel_spmd(nc, in_maps, list(range(8)))
    LAST_RUN_S = time.time() - t0
    out = np.concatenate([
        np.asarray(res.results[2 * b]["out"], np.float32).reshape(-1)
        for b in range(B)])
    return out

